# revision 7
# baseline (speedup 1.0000x reference)
"""Self-contained Trainium2 Bass kernel for the GCN encoder layer
(GCNConv + PReLU), distributed over 8 NeuronCores.

    out = PReLU(A_hat @ x @ W + b),  A_hat = D^-1/2 (A + I) D^-1/2

Strategy v3 (bf16 streaming, host does partitioning/indexing/layout):
  * Destination nodes are sharded across the 8 cores (12500 rows each).
  * Aggregation runs in x-space first (agg = A_hat @ x), then the dense
    transform (agg @ W) per 8-bin group (1024 dsts, 2 PSUM banks).
  * Per core, destinations are packed into bins of 128 (balancing
    packer); each bin owns 6 or 7 static 128-edge tiles holding only the
    real edges. Self-loop contributions are folded in at the epilogue
    via a second accumulating W-matmul against xself (host-prepared
    dis^2-scaled own rows, transposed), avoiding ~100 tiles of work.
  * The host pre-gathers x[src] per slot into a partition-major bf16
    "image" xg[p, g*C:(g+1)*C] = x[src(g,p)], so the device streams it
    with large contiguous DMA descriptors (4KB/partition/chunk) instead
    of per-edge 512B gather descriptors. bf16 halves the traffic.
  * Per tile, a scaled one-hot S[e, j] = (j == dstoff_e) * norm_e is
    built with one tensor_scalar (iota compare, bf16 in/out for the 4x
    DVE mode), split ~2:1 between the DVE and the otherwise-idle Pool
    (gpsimd) engine; the segment sum runs on the tensor engine:
    aggT[c, d] += Xg^T @ S, accumulated in PSUM (bf16 matmuls run at
    1 cycle/row vs 4 for fp32).
  * Epilogue per 8-bin group: one Copy (PSUM->SBUF, cast bf16), W
    matmuls accumulating W^T @ (aggTs | xself), two Relu activations
    (per-partition bias) and one DVE scalar_tensor_tensor for PReLU.
  * out_t [128 ch, 12544 dst] bf16 per core; host upcasts, transposes,
    and un-permutes.
"""

import numpy as np
import ml_dtypes

import concourse.bass as bass
import concourse.bacc as bacc
import concourse.tile as tile
import concourse.mybir as mybir
from concourse.bass_utils import run_bass_kernel_spmd

F32 = mybir.dt.float32
BF16 = mybir.dt.bfloat16
NPBF16 = ml_dtypes.bfloat16

N = 100000
C = 128
P = 128
NCORES = 8
PER = N // NCORES            # 12500
NBINS = (PER + P - 1) // P   # 98
DPAD = NBINS * P             # 12544
GRP = 8                      # bins per epilogue group (1024 dsts, 2 banks)
CH_TILES = 32                # tiles per DMA chunk (8KB/partition/descriptor)
POOL_EVERY = 2               # every 2nd S-build goes to the Pool engine


# ----------------------------------------------------------------------
# host-side preprocessing
# ----------------------------------------------------------------------

def _pack_core_capped(deg_local, caps):
    """Assign PER dsts (+ pads) to NBINS bins of exactly P dsts with
    per-bin edge capacity caps. Feasibility-aware balancing greedy."""
    npad = DPAD - len(deg_local)
    deg_all = np.concatenate([deg_local,
                              np.zeros(npad, dtype=deg_local.dtype)])
    order = np.argsort(-deg_all, kind="stable")
    slots_left = np.full(NBINS, P, dtype=np.int64)
    cap_left = np.asarray(caps, dtype=np.int64).copy()
    bin_of = np.empty(DPAD, dtype=np.int64)
    pos_of = np.empty(DPAD, dtype=np.int64)
    for dd in order:
        dg = deg_all[dd]
        ok = (slots_left > 0) & (cap_left >= dg)
        if not ok.any():
            return None
        cand = np.where(ok)[0]
        bb = cand[np.argmax(cap_left[cand] / slots_left[cand])]
        bin_of[dd] = bb
        pos_of[dd] = P - slots_left[bb]
        slots_left[bb] -= 1
        cap_left[bb] -= dg
    return bin_of, pos_of


def _build_all(src, dst):
    deg = np.bincount(dst, minlength=N).astype(np.int64) + 1
    dis = 1.0 / np.sqrt(deg.astype(np.float64))
    core_of = dst // PER
    rdeg = deg - 1                     # real (non-self) in-degree

    base_t = 6
    khi = []
    for c in range(NCORES):
        rdeg_c = rdeg[c * PER:(c + 1) * PER]
        edges_c = int(rdeg_c.sum())
        khi.append(max(0, -(-(edges_c - NBINS * base_t * P) // P)) + 2)
    KHI = max(khi)
    packs = None
    while packs is None and KHI <= NBINS:
        caps = np.array([(base_t + 1) * P] * KHI +
                        [base_t * P] * (NBINS - KHI), dtype=np.int64)
        packs = []
        for c in range(NCORES):
            r = _pack_core_capped(rdeg[c * PER:(c + 1) * PER], caps)
            if r is None:
                packs = None
                KHI += 2
                break
            packs.append(r)
    assert packs is not None, "bin packing failed"
    tiles_of_bin = np.array([base_t + 1] * KHI +
                            [base_t] * (NBINS - KHI), dtype=np.int64)
    G = int(tiles_of_bin.sum())
    tile_base = np.concatenate([[0], np.cumsum(tiles_of_bin)])[:-1]

    static = dict(tiles_of_bin=tiles_of_bin, tile_base=tile_base, G=G)

    cores = []
    for c in range(NCORES):
        bin_of, pos_of = packs[c]
        mask = core_of == c
        all_src = src[mask]
        all_dstl = dst[mask] - c * PER

        b_of_e = bin_of[all_dstl]
        order = np.argsort(b_of_e, kind="stable")
        s_sorted = all_src[order]
        b_sorted = b_of_e[order]

        counts = np.bincount(b_sorted, minlength=NBINS)
        run_start = np.concatenate([[0], np.cumsum(counts)])[:-1]
        within = np.arange(len(b_sorted)) - run_start[b_sorted]
        g = tile_base[b_sorted] + within // P
        p = within % P
        slot = g * P + p

        srcidx = np.zeros(G * P, dtype=np.int64)
        normv = np.zeros(G * P, dtype=np.float64)
        dofv = np.zeros(G * P, dtype=np.float64)
        srcidx[slot] = s_sorted
        all_dst_global = all_dstl[order] + c * PER
        normv[slot] = dis[s_sorted] * dis[all_dst_global]
        dofv[slot] = pos_of[all_dstl[order]].astype(np.float64)

        norm = normv.reshape(G, P).T.astype(np.float32).copy()
        dof = dofv.reshape(G, P).T.astype(np.float32).copy()
        outrow_of_dst = bin_of * P + pos_of
        dis2 = (dis[c * PER:(c + 1) * PER] ** 2).astype(np.float64)
        cores.append(dict(srcidx=srcidx.reshape(G, P), norm=norm, dof=dof,
                          outrow_of_dst=outrow_of_dst, dis2=dis2))
    return static, cores


# ----------------------------------------------------------------------
# device program
# ----------------------------------------------------------------------

def _build_program(static, repeat=1):
    tiles_of_bin = static["tiles_of_bin"]
    G = static["G"]

    nc = bacc.Bacc("TRN2", target_bir_lowering=False, debug=False,
                   num_devices=NCORES)

    xg_d = nc.dram_tensor("xg", [P, G * C], BF16, kind="ExternalInput")
    xs_d = nc.dram_tensor("xself", [C, DPAD], BF16, kind="ExternalInput")
    nm_d = nc.dram_tensor("edgenorm", [P, G], F32, kind="ExternalInput")
    do_d = nc.dram_tensor("dstoff", [P, G], F32, kind="ExternalInput")
    w_d = nc.dram_tensor("Wt", [C, C], BF16, kind="ExternalInput")
    bias_d = nc.dram_tensor("bias", [C, 1], F32, kind="ExternalInput")
    nbias_d = nc.dram_tensor("nbias", [C, 1], F32, kind="ExternalInput")
    nalpha_d = nc.dram_tensor("nalpha", [C, 1], F32, kind="ExternalInput")
    nab_d = nc.dram_tensor("nab", [C, 1], F32, kind="ExternalInput")
    iota_d = nc.dram_tensor("iota", [P, P], BF16, kind="ExternalInput")
    out_d = nc.dram_tensor("out_t", [C, DPAD], BF16, kind="ExternalOutput")

    groups = []
    b0 = 0
    while b0 < NBINS:
        left = NBINS - b0
        gw = GRP if left > 10 else (4 if left > 6 else 2)
        gw = min(gw, left)
        groups.append(list(range(b0, b0 + gw)))
        b0 += gw

    with tile.TileContext(nc) as tc:
        with (
            tc.tile_pool(name="const", bufs=1) as constp,
            tc.tile_pool(name="xg", bufs=6) as xgp,
            tc.tile_pool(name="xs", bufs=3) as xsp,
            tc.tile_pool(name="s", bufs=10) as sp,
            tc.tile_pool(name="aggts", bufs=3) as aggp,
            tc.tile_pool(name="res", bufs=6) as resp,
            tc.tile_pool(name="psA", bufs=2, space="PSUM") as psA,
            tc.tile_pool(name="psB", bufs=2, space="PSUM") as psB,
        ):
            w_sb = constp.tile([C, C], BF16)
            iota_sb = constp.tile([P, P], BF16)
            b_sb = constp.tile([C, 1], F32)
            nb_sb = constp.tile([C, 1], F32)
            na_sb = constp.tile([C, 1], F32)
            nab_sb = constp.tile([C, 1], F32)
            nrm_sb = constp.tile([P, G], F32)
            dof_sb = constp.tile([P, G], F32)
            nc.sync.dma_start(out=w_sb[:], in_=w_d[:, :])
            nc.sync.dma_start(out=iota_sb[:], in_=iota_d[:, :])
            nc.sync.dma_start(out=b_sb[:], in_=bias_d[:, :])
            nc.sync.dma_start(out=nb_sb[:], in_=nbias_d[:, :])
            nc.sync.dma_start(out=na_sb[:], in_=nalpha_d[:, :])
            nc.sync.dma_start(out=nab_sb[:], in_=nab_d[:, :])
            nc.sync.dma_start(out=nrm_sb[:], in_=nm_d[:, :])
            nc.sync.dma_start(out=dof_sb[:], in_=do_d[:, :])

            # chunk plan: big chunks, tapering at the end to shrink the
            # pipeline drain after the final chunk lands
            bounds = [0]
            while bounds[-1] < G:
                left = G - bounds[-1]
                if left > 32:
                    step = CH_TILES
                elif left > 8:
                    step = 8
                else:
                    step = 4
                bounds.append(min(bounds[-1] + step, G))
            chunk_of_tile = np.zeros(G, dtype=np.int64)
            for ci in range(len(bounds) - 1):
                chunk_of_tile[bounds[ci]:bounds[ci + 1]] = ci

            cur = {}

            def load_chunk(ci):
                g0, g1 = bounds[ci], bounds[ci + 1]
                xg = xgp.tile([P, CH_TILES * C], BF16, tag="xg")
                nc.sync.dma_start(out=xg[:, :(g1 - g0) * C],
                                  in_=xg_d[:, g0 * C:g1 * C])
                cur[ci] = (xg, g0)

            for _rep in range(repeat):
                cur.clear()
                g = 0
                for group in groups:
                    gw = len(group)
                    b0 = group[0]
                    agg = psA.tile([C, GRP * P], F32, tag="agg")
                    xs_sb = xsp.tile([C, GRP * P], BF16, tag="xs")
                    nc.sync.dma_start(
                        out=xs_sb[:, :gw * P],
                        in_=xs_d[:, b0 * P:b0 * P + gw * P])
                    for j, b in enumerate(group):
                        T = int(tiles_of_bin[b])
                        for t in range(T):
                            ci = int(chunk_of_tile[g])
                            if ci not in cur:
                                load_chunk(ci)
                            xg, g0 = cur[ci]
                            k = g - g0
                            S = sp.tile([P, P], BF16, tag="S")
                            eng = (nc.gpsimd if (g % POOL_EVERY
                                                 == POOL_EVERY - 1)
                                   else nc.vector)
                            eng.tensor_scalar(
                                out=S[:],
                                in0=iota_sb[:],
                                scalar1=dof_sb[:, g:g + 1],
                                scalar2=nrm_sb[:, g:g + 1],
                                op0=mybir.AluOpType.is_equal,
                                op1=mybir.AluOpType.mult,
                            )
                            nc.tensor.matmul(
                                out=agg[:, j * P:(j + 1) * P],
                                lhsT=xg[:, k * C:(k + 1) * C],
                                rhs=S[:],
                                start=(t == 0),
                                stop=(t == T - 1),
                            )
                            g += 1
                    aggTs = aggp.tile([C, GRP * P], BF16, tag="aggTs")
                    nc.scalar.activation(
                        out=aggTs[:, :gw * P], in_=agg[:, :gw * P],
                        func=mybir.ActivationFunctionType.Copy,
                    )
                    out2 = psB.tile([C, GRP * P], F32, tag="out2")
                    for h0 in range(0, gw, 4):
                        hw = min(4, gw - h0) * P
                        sl = slice(h0 * P, h0 * P + hw)
                        nc.tensor.matmul(out=out2[:, sl], lhsT=w_sb[:],
                                         rhs=aggTs[:, sl],
                                         start=True, stop=False)
                        nc.tensor.matmul(out=out2[:, sl], lhsT=w_sb[:],
                                         rhs=xs_sb[:, sl],
                                         start=False, stop=True)
                    pos = resp.tile([C, GRP * P], BF16, tag="pos")
                    nc.scalar.activation(
                        out=pos[:, :gw * P], in_=out2[:, :gw * P],
                        func=mybir.ActivationFunctionType.Relu,
                        bias=b_sb[:, :1], scale=1.0,
                    )
                    neg = resp.tile([C, GRP * P], BF16, tag="neg")
                    nc.scalar.activation(
                        out=neg[:, :gw * P], in_=out2[:, :gw * P],
                        func=mybir.ActivationFunctionType.Relu,
                        bias=nab_sb[:, :1], scale=na_sb[:, :1],
                    )
                    res = resp.tile([C, GRP * P], BF16, tag="res")
                    nc.vector.tensor_tensor(
                        out=res[:, :gw * P],
                        in0=pos[:, :gw * P],
                        in1=neg[:, :gw * P],
                        op=mybir.AluOpType.subtract,
                    )
                    nc.sync.dma_start(
                        out=out_d[:, b0 * P:b0 * P + gw * P],
                        in_=res[:, :gw * P])

    nc.compile()
    return nc


# ----------------------------------------------------------------------
# public entry point
# ----------------------------------------------------------------------

_CACHE = {}


def _get_compiled(src, dst):
    key = hash((src.tobytes(), dst.tobytes()))
    if key not in _CACHE:
        static, cores = _build_all(src, dst)
        nc = _build_program(static)
        _CACHE[key] = (static, cores, nc)
    return _CACHE[key]


def _make_in_maps(static, cores, x, W, b, prelu_w):
    G = static["G"]
    x_bf = x.astype(NPBF16)
    iota = np.tile(np.arange(P, dtype=NPBF16), (P, 1))
    in_maps = []
    for c, ca in enumerate(cores):
        # partition-major image: xg_img[p, g*C:(g+1)*C] = x[src(g, p)]
        xg = x_bf[ca["srcidx"]]                     # [G, P, C]
        xg_img = np.ascontiguousarray(
            xg.transpose(1, 0, 2)).reshape(P, G * C)
        # self-loop rows, dis^2-scaled, in out_t column order, transposed
        xs = np.zeros((DPAD, C), dtype=np.float64)
        rows = ca["outrow_of_dst"][:PER]
        xs[rows] = x[c * PER:(c + 1) * PER].astype(np.float64) \
            * ca["dis2"][:, None]
        xself_t = np.ascontiguousarray(xs.T.astype(NPBF16))
        in_maps.append({
            "xg": xg_img,
            "xself": xself_t,
            "edgenorm": ca["norm"],
            "dstoff": ca["dof"],
            "Wt": W.astype(NPBF16),
            "bias": b.reshape(C, 1).astype(np.float32),
            "nbias": (-b).reshape(C, 1).astype(np.float32),
            "nalpha": (-prelu_w).reshape(C, 1).astype(np.float32),
            "nab": (-prelu_w * b).reshape(C, 1).astype(np.float32),
            "iota": iota,
        })
    return in_maps


def kernel(x, edge_index, W, b, prelu_w):
    x = np.ascontiguousarray(np.asarray(x, dtype=np.float32))
    ei = np.asarray(edge_index)
    W = np.asarray(W, dtype=np.float32)
    b = np.asarray(b, dtype=np.float32)
    prelu_w = np.asarray(prelu_w, dtype=np.float32)
    src = ei[0].astype(np.int64)
    dst = ei[1].astype(np.int64)
    assert x.shape == (N, C), x.shape

    static, cores, nc = _get_compiled(src, dst)
    in_maps = _make_in_maps(static, cores, x, W, b, prelu_w)

    res = None
    for attempt in range(3):
        try:
            res = run_bass_kernel_spmd(nc, in_maps,
                                       core_ids=list(range(NCORES)))
            break
        except Exception:
            if attempt == 2:
                raise
            import time as _time
            _time.sleep(20.0)

    out = np.empty((N, C), dtype=np.float32)
    for c, ca in enumerate(cores):
        ot = res.results[c]["out_t"].astype(np.float32)   # [C, DPAD]
        oc = np.ascontiguousarray(ot.T)                   # [DPAD, C]
        out[c * PER:(c + 1) * PER] = oc[ca["outrow_of_dst"][:PER]]
    return out


# revision 8
# speedup vs baseline: 6.2016x; 6.2016x over previous
"""Self-contained Trainium2 Bass kernel for the GCN encoder layer
(GCNConv + PReLU), distributed over 8 NeuronCores.

    out = PReLU(A_hat @ x @ W + b),  A_hat = D^-1/2 (A + I) D^-1/2

Strategy v3 (bf16 streaming, host does partitioning/indexing/layout):
  * Destination nodes are sharded across the 8 cores (12500 rows each).
  * Aggregation runs in x-space first (agg = A_hat @ x), then the dense
    transform (agg @ W) per 8-bin group (1024 dsts, 2 PSUM banks).
  * Per core, destinations are packed into bins of 128 (balancing
    packer); each bin owns 6 or 7 static 128-edge tiles holding only the
    real edges. Self-loop contributions are folded in at the epilogue
    via a second accumulating W-matmul against xself (host-prepared
    dis^2-scaled own rows, transposed), avoiding ~100 tiles of work.
  * The host pre-gathers x[src] per slot into a partition-major bf16
    "image" xg[p, g*C:(g+1)*C] = x[src(g,p)], so the device streams it
    with large contiguous DMA descriptors (4KB/partition/chunk) instead
    of per-edge 512B gather descriptors. bf16 halves the traffic.
  * Per tile, a scaled one-hot S[e, j] = (j == dstoff_e) * norm_e is
    built with one tensor_scalar (iota compare, bf16 in/out for the 4x
    DVE mode), split ~2:1 between the DVE and the otherwise-idle Pool
    (gpsimd) engine; the segment sum runs on the tensor engine:
    aggT[c, d] += Xg^T @ S, accumulated in PSUM (bf16 matmuls run at
    1 cycle/row vs 4 for fp32).
  * Epilogue per 8-bin group: one Copy (PSUM->SBUF, cast bf16), W
    matmuls accumulating W^T @ (aggTs | xself), two Relu activations
    (per-partition bias) and one DVE scalar_tensor_tensor for PReLU.
  * out_t [128 ch, 12544 dst] bf16 per core; host upcasts, transposes,
    and un-permutes.
"""

import numpy as np
import ml_dtypes

import concourse.bass as bass
import concourse.bacc as bacc
import concourse.tile as tile
import concourse.mybir as mybir
from concourse.bass_utils import run_bass_kernel_spmd

F32 = mybir.dt.float32
BF16 = mybir.dt.bfloat16
NPBF16 = ml_dtypes.bfloat16

N = 100000
C = 128
P = 128
NCORES = 8
PER = N // NCORES            # 12500
NBINS = (PER + P - 1) // P   # 98
DPAD = NBINS * P             # 12544
GRP = 8                      # bins per epilogue group (1024 dsts, 2 banks)
CH_TILES = 32                # tiles per DMA chunk (8KB/partition/descriptor)
POOL_EVERY = 10 ** 9         # Pool tensor_scalar is slow on real HW; keep S-builds on DVE


# ----------------------------------------------------------------------
# host-side preprocessing
# ----------------------------------------------------------------------

def _pack_core_capped(deg_local, caps):
    """Assign PER dsts (+ pads) to NBINS bins of exactly P dsts with
    per-bin edge capacity caps. Feasibility-aware balancing greedy."""
    npad = DPAD - len(deg_local)
    deg_all = np.concatenate([deg_local,
                              np.zeros(npad, dtype=deg_local.dtype)])
    order = np.argsort(-deg_all, kind="stable")
    slots_left = np.full(NBINS, P, dtype=np.int64)
    cap_left = np.asarray(caps, dtype=np.int64).copy()
    bin_of = np.empty(DPAD, dtype=np.int64)
    pos_of = np.empty(DPAD, dtype=np.int64)
    for dd in order:
        dg = deg_all[dd]
        ok = (slots_left > 0) & (cap_left >= dg)
        if not ok.any():
            return None
        cand = np.where(ok)[0]
        bb = cand[np.argmax(cap_left[cand] / slots_left[cand])]
        bin_of[dd] = bb
        pos_of[dd] = P - slots_left[bb]
        slots_left[bb] -= 1
        cap_left[bb] -= dg
    return bin_of, pos_of


def _build_all(src, dst):
    deg = np.bincount(dst, minlength=N).astype(np.int64) + 1
    dis = 1.0 / np.sqrt(deg.astype(np.float64))
    core_of = dst // PER
    rdeg = deg - 1                     # real (non-self) in-degree

    base_t = 6
    khi = []
    for c in range(NCORES):
        rdeg_c = rdeg[c * PER:(c + 1) * PER]
        edges_c = int(rdeg_c.sum())
        khi.append(max(0, -(-(edges_c - NBINS * base_t * P) // P)) + 2)
    KHI = max(khi)
    packs = None
    while packs is None and KHI <= NBINS:
        caps = np.array([(base_t + 1) * P] * KHI +
                        [base_t * P] * (NBINS - KHI), dtype=np.int64)
        packs = []
        for c in range(NCORES):
            r = _pack_core_capped(rdeg[c * PER:(c + 1) * PER], caps)
            if r is None:
                packs = None
                KHI += 2
                break
            packs.append(r)
    assert packs is not None, "bin packing failed"
    tiles_of_bin = np.array([base_t + 1] * KHI +
                            [base_t] * (NBINS - KHI), dtype=np.int64)
    G = int(tiles_of_bin.sum())
    tile_base = np.concatenate([[0], np.cumsum(tiles_of_bin)])[:-1]

    static = dict(tiles_of_bin=tiles_of_bin, tile_base=tile_base, G=G)

    cores = []
    for c in range(NCORES):
        bin_of, pos_of = packs[c]
        mask = core_of == c
        all_src = src[mask]
        all_dstl = dst[mask] - c * PER

        b_of_e = bin_of[all_dstl]
        order = np.argsort(b_of_e, kind="stable")
        s_sorted = all_src[order]
        b_sorted = b_of_e[order]

        counts = np.bincount(b_sorted, minlength=NBINS)
        run_start = np.concatenate([[0], np.cumsum(counts)])[:-1]
        within = np.arange(len(b_sorted)) - run_start[b_sorted]
        g = tile_base[b_sorted] + within // P
        p = within % P
        slot = g * P + p

        srcidx = np.zeros(G * P, dtype=np.int64)
        normv = np.zeros(G * P, dtype=np.float64)
        dofv = np.zeros(G * P, dtype=np.float64)
        srcidx[slot] = s_sorted
        all_dst_global = all_dstl[order] + c * PER
        normv[slot] = dis[s_sorted] * dis[all_dst_global]
        dofv[slot] = pos_of[all_dstl[order]].astype(np.float64)

        norm = normv.reshape(G, P).T.astype(np.float32).copy()
        dof = dofv.reshape(G, P).T.astype(np.float32).copy()
        outrow_of_dst = bin_of * P + pos_of
        dis2 = (dis[c * PER:(c + 1) * PER] ** 2).astype(np.float64)
        cores.append(dict(srcidx=srcidx.reshape(G, P), norm=norm, dof=dof,
                          outrow_of_dst=outrow_of_dst, dis2=dis2))
    return static, cores


# ----------------------------------------------------------------------
# device program
# ----------------------------------------------------------------------

def _build_program(static, repeat=1):
    tiles_of_bin = static["tiles_of_bin"]
    G = static["G"]

    nc = bacc.Bacc("TRN2", target_bir_lowering=False, debug=False,
                   num_devices=NCORES)

    xg_d = nc.dram_tensor("xg", [P, G * C], BF16, kind="ExternalInput")
    xs_d = nc.dram_tensor("xself", [C, DPAD], BF16, kind="ExternalInput")
    nm_d = nc.dram_tensor("edgenorm", [P, G], F32, kind="ExternalInput")
    do_d = nc.dram_tensor("dstoff", [P, G], F32, kind="ExternalInput")
    w_d = nc.dram_tensor("Wt", [C, C], BF16, kind="ExternalInput")
    bias_d = nc.dram_tensor("bias", [C, 1], F32, kind="ExternalInput")
    nbias_d = nc.dram_tensor("nbias", [C, 1], F32, kind="ExternalInput")
    nalpha_d = nc.dram_tensor("nalpha", [C, 1], F32, kind="ExternalInput")
    nab_d = nc.dram_tensor("nab", [C, 1], F32, kind="ExternalInput")
    iota_d = nc.dram_tensor("iota", [P, P], BF16, kind="ExternalInput")
    out_d = nc.dram_tensor("out_t", [C, DPAD], BF16, kind="ExternalOutput")

    groups = []
    b0 = 0
    while b0 < NBINS:
        left = NBINS - b0
        gw = GRP if left > 10 else (4 if left > 6 else 2)
        gw = min(gw, left)
        groups.append(list(range(b0, b0 + gw)))
        b0 += gw

    with tile.TileContext(nc) as tc:
        with (
            tc.tile_pool(name="const", bufs=1) as constp,
            tc.tile_pool(name="xg", bufs=6) as xgp,
            tc.tile_pool(name="xs", bufs=3) as xsp,
            tc.tile_pool(name="s", bufs=10) as sp,
            tc.tile_pool(name="aggts", bufs=3) as aggp,
            tc.tile_pool(name="res", bufs=6) as resp,
            tc.tile_pool(name="psA", bufs=2, space="PSUM") as psA,
            tc.tile_pool(name="psB", bufs=2, space="PSUM") as psB,
        ):
            w_sb = constp.tile([C, C], BF16)
            iota_sb = constp.tile([P, P], BF16)
            b_sb = constp.tile([C, 1], F32)
            nb_sb = constp.tile([C, 1], F32)
            na_sb = constp.tile([C, 1], F32)
            nab_sb = constp.tile([C, 1], F32)
            nrm_sb = constp.tile([P, G], F32)
            dof_sb = constp.tile([P, G], F32)
            nc.sync.dma_start(out=w_sb[:], in_=w_d[:, :])
            nc.sync.dma_start(out=iota_sb[:], in_=iota_d[:, :])
            nc.sync.dma_start(out=b_sb[:], in_=bias_d[:, :])
            nc.sync.dma_start(out=nb_sb[:], in_=nbias_d[:, :])
            nc.sync.dma_start(out=na_sb[:], in_=nalpha_d[:, :])
            nc.sync.dma_start(out=nab_sb[:], in_=nab_d[:, :])
            nc.sync.dma_start(out=nrm_sb[:], in_=nm_d[:, :])
            nc.sync.dma_start(out=dof_sb[:], in_=do_d[:, :])

            # chunk plan: big chunks, tapering at the end to shrink the
            # pipeline drain after the final chunk lands
            bounds = [0]
            while bounds[-1] < G:
                left = G - bounds[-1]
                if left > 32:
                    step = CH_TILES
                elif left > 8:
                    step = 8
                else:
                    step = 4
                bounds.append(min(bounds[-1] + step, G))
            chunk_of_tile = np.zeros(G, dtype=np.int64)
            for ci in range(len(bounds) - 1):
                chunk_of_tile[bounds[ci]:bounds[ci + 1]] = ci

            cur = {}

            def load_chunk(ci):
                g0, g1 = bounds[ci], bounds[ci + 1]
                xg = xgp.tile([P, CH_TILES * C], BF16, tag="xg")
                nc.sync.dma_start(out=xg[:, :(g1 - g0) * C],
                                  in_=xg_d[:, g0 * C:g1 * C])
                cur[ci] = (xg, g0)

            for _rep in range(repeat):
                cur.clear()
                g = 0
                for group in groups:
                    gw = len(group)
                    b0 = group[0]
                    agg = psA.tile([C, GRP * P], F32, tag="agg")
                    xs_sb = xsp.tile([C, GRP * P], BF16, tag="xs")
                    nc.sync.dma_start(
                        out=xs_sb[:, :gw * P],
                        in_=xs_d[:, b0 * P:b0 * P + gw * P])
                    for j, b in enumerate(group):
                        T = int(tiles_of_bin[b])
                        for t in range(T):
                            ci = int(chunk_of_tile[g])
                            if ci not in cur:
                                load_chunk(ci)
                            xg, g0 = cur[ci]
                            k = g - g0
                            S = sp.tile([P, P], BF16, tag="S")
                            eng = (nc.gpsimd if (g % POOL_EVERY
                                                 == POOL_EVERY - 1)
                                   else nc.vector)
                            eng.tensor_scalar(
                                out=S[:],
                                in0=iota_sb[:],
                                scalar1=dof_sb[:, g:g + 1],
                                scalar2=nrm_sb[:, g:g + 1],
                                op0=mybir.AluOpType.is_equal,
                                op1=mybir.AluOpType.mult,
                            )
                            nc.tensor.matmul(
                                out=agg[:, j * P:(j + 1) * P],
                                lhsT=xg[:, k * C:(k + 1) * C],
                                rhs=S[:],
                                start=(t == 0),
                                stop=(t == T - 1),
                            )
                            g += 1
                    aggTs = aggp.tile([C, GRP * P], BF16, tag="aggTs")
                    nc.scalar.activation(
                        out=aggTs[:, :gw * P], in_=agg[:, :gw * P],
                        func=mybir.ActivationFunctionType.Copy,
                    )
                    out2 = psB.tile([C, GRP * P], F32, tag="out2")
                    for h0 in range(0, gw, 4):
                        hw = min(4, gw - h0) * P
                        sl = slice(h0 * P, h0 * P + hw)
                        nc.tensor.matmul(out=out2[:, sl], lhsT=w_sb[:],
                                         rhs=aggTs[:, sl],
                                         start=True, stop=False)
                        nc.tensor.matmul(out=out2[:, sl], lhsT=w_sb[:],
                                         rhs=xs_sb[:, sl],
                                         start=False, stop=True)
                    pos = resp.tile([C, GRP * P], BF16, tag="pos")
                    nc.scalar.activation(
                        out=pos[:, :gw * P], in_=out2[:, :gw * P],
                        func=mybir.ActivationFunctionType.Relu,
                        bias=b_sb[:, :1], scale=1.0,
                    )
                    neg = resp.tile([C, GRP * P], BF16, tag="neg")
                    nc.scalar.activation(
                        out=neg[:, :gw * P], in_=out2[:, :gw * P],
                        func=mybir.ActivationFunctionType.Relu,
                        bias=nab_sb[:, :1], scale=na_sb[:, :1],
                    )
                    res = resp.tile([C, GRP * P], BF16, tag="res")
                    nc.vector.tensor_tensor(
                        out=res[:, :gw * P],
                        in0=pos[:, :gw * P],
                        in1=neg[:, :gw * P],
                        op=mybir.AluOpType.subtract,
                    )
                    nc.sync.dma_start(
                        out=out_d[:, b0 * P:b0 * P + gw * P],
                        in_=res[:, :gw * P])

    nc.compile()
    return nc


# ----------------------------------------------------------------------
# public entry point
# ----------------------------------------------------------------------

_CACHE = {}


def _get_compiled(src, dst):
    key = hash((src.tobytes(), dst.tobytes()))
    if key not in _CACHE:
        static, cores = _build_all(src, dst)
        nc = _build_program(static)
        _CACHE[key] = (static, cores, nc)
    return _CACHE[key]


def _make_in_maps(static, cores, x, W, b, prelu_w):
    G = static["G"]
    x_bf = x.astype(NPBF16)
    iota = np.tile(np.arange(P, dtype=NPBF16), (P, 1))
    in_maps = []
    for c, ca in enumerate(cores):
        # partition-major image: xg_img[p, g*C:(g+1)*C] = x[src(g, p)]
        xg = x_bf[ca["srcidx"]]                     # [G, P, C]
        xg_img = np.ascontiguousarray(
            xg.transpose(1, 0, 2)).reshape(P, G * C)
        # self-loop rows, dis^2-scaled, in out_t column order, transposed
        xs = np.zeros((DPAD, C), dtype=np.float64)
        rows = ca["outrow_of_dst"][:PER]
        xs[rows] = x[c * PER:(c + 1) * PER].astype(np.float64) \
            * ca["dis2"][:, None]
        xself_t = np.ascontiguousarray(xs.T.astype(NPBF16))
        in_maps.append({
            "xg": xg_img,
            "xself": xself_t,
            "edgenorm": ca["norm"],
            "dstoff": ca["dof"],
            "Wt": W.astype(NPBF16),
            "bias": b.reshape(C, 1).astype(np.float32),
            "nbias": (-b).reshape(C, 1).astype(np.float32),
            "nalpha": (-prelu_w).reshape(C, 1).astype(np.float32),
            "nab": (-prelu_w * b).reshape(C, 1).astype(np.float32),
            "iota": iota,
        })
    return in_maps


def kernel(x, edge_index, W, b, prelu_w):
    x = np.ascontiguousarray(np.asarray(x, dtype=np.float32))
    ei = np.asarray(edge_index)
    W = np.asarray(W, dtype=np.float32)
    b = np.asarray(b, dtype=np.float32)
    prelu_w = np.asarray(prelu_w, dtype=np.float32)
    src = ei[0].astype(np.int64)
    dst = ei[1].astype(np.int64)
    assert x.shape == (N, C), x.shape

    static, cores, nc = _get_compiled(src, dst)
    in_maps = _make_in_maps(static, cores, x, W, b, prelu_w)

    res = None
    for attempt in range(3):
        try:
            res = run_bass_kernel_spmd(nc, in_maps,
                                       core_ids=list(range(NCORES)))
            break
        except Exception:
            if attempt == 2:
                raise
            import time as _time
            _time.sleep(20.0)

    out = np.empty((N, C), dtype=np.float32)
    for c, ca in enumerate(cores):
        ot = res.results[c]["out_t"].astype(np.float32)   # [C, DPAD]
        oc = np.ascontiguousarray(ot.T)                   # [DPAD, C]
        out[c * PER:(c + 1) * PER] = oc[ca["outrow_of_dst"][:PER]]
    return out
